# revision 1
# baseline (speedup 1.0000x reference)
"""BERT-style dense transformer kernel for 8 Trainium2 NeuronCores.

Data-parallel over batch (B=4096 -> 512/core). Per core:
  embed (per-column [V,H] matmul) -> 4 transformer layers -> per-column head
  + log_softmax. Token-major master layout [128 tokens, H] with feature-major
  side tensors produced via PE transposes where matmuls need them as lhsT.
Matmul inputs bf16 (fp32 PSUM accumulation); residual/LN/softmax in fp32.
"""
import sys
sys.path.insert(0, '/opt/trn_rl_repo')
import numpy as np
import ml_dtypes

import concourse.bass as bass
import concourse.bacc as bacc
import concourse.tile as tile
from concourse import mybir
from concourse.bass_utils import run_bass_kernel_spmd
from concourse.masks import make_identity

F32, BF16, F32R = mybir.dt.float32, mybir.dt.bfloat16, mybir.dt.float32r
AF = mybir.ActivationFunctionType
ALU = mybir.AluOpType
AX = mybir.AxisListType
BF16NP = ml_dtypes.bfloat16

# Problem constants
B, C, V, H, NH, L = 4096, 16, 1000, 512, 8, 4
DK = H // NH          # 64
FF = 4 * H            # 2048
NCORES = 8
BS = B // NCORES      # 512 batch/core
T = BS * C            # 8192 tokens/core
P = 128
NT = T // P           # 64 token tiles
HC = H // P           # 4 feature chunks
FC = FF // P          # 16 ff chunks
SCALE = 1.0 / np.sqrt(DK)  # 0.125
NEG_BIG = -1e9
NEG_HUGE = -3e38
# V (=1000) contraction chunks for embed: 7 full 128-chunks + tail 104
VCH = [(i * 128, 128) for i in range(7)] + [(896, 104)]

_CACHED = {}
DEBUG = False


def build_kernel():
    nc = bacc.Bacc(None)

    xTin = nc.dram_tensor("xTin", [C, V, BS], BF16, kind="ExternalInput")
    embW = nc.dram_tensor("embW", [C, V, H], BF16, kind="ExternalInput")
    wq = nc.dram_tensor("wq", [L, H, H], BF16, kind="ExternalInput")
    wk = nc.dram_tensor("wk", [L, H, H], BF16, kind="ExternalInput")
    wv = nc.dram_tensor("wv", [L, H, H], BF16, kind="ExternalInput")
    wo = nc.dram_tensor("wo", [L, H, H], BF16, kind="ExternalInput")
    w1 = nc.dram_tensor("w1", [L, H, FF], BF16, kind="ExternalInput")
    w2 = nc.dram_tensor("w2", [L, FF, H], BF16, kind="ExternalInput")
    headW = nc.dram_tensor("headW", [C, H, V], BF16, kind="ExternalInput")
    uemb = nc.dram_tensor("uemb", [C, BS // P, P, 1], F32, kind="ExternalInput")
    w15emb = nc.dram_tensor("w15emb", [C, BS // P, P, 1], F32, kind="ExternalInput")
    vmask = nc.dram_tensor("vmask", [NT, P, P], BF16, kind="ExternalInput")
    amask = nc.dram_tensor("amask", [NT, P, P], BF16, kind="ExternalInput")
    out = nc.dram_tensor("out", [BS, C, V], F32, kind="ExternalOutput")
    dbg = nc.dram_tensor("dbg", [1 + L, T, H], F32, kind="ExternalOutput") if DEBUG else None

    xbuf = nc.dram_tensor("xbuf", [T, H], F32)
    x_c = xbuf.rearrange("(n c) h -> n c h", c=C)  # [BS, C, H] token rows by (b, c)

    with tile.TileContext(nc) as tc:
        # ---------------- constants ----------------
        const_cm = tc.tile_pool(name="const", bufs=1)
        const = const_cm.__enter__()
        ident = const.tile([P, P], BF16)
        make_identity(nc, ident[:])
        eps_t = const.tile([P, 1], F32)
        nc.vector.memset(eps_t[:], 1e-6)

        # ---------------- embed phase ----------------
        with tc.tile_pool(name="e_w", bufs=2) as e_w, \
             tc.tile_pool(name="e_x", bufs=3) as e_x, \
             tc.tile_pool(name="e_sc", bufs=3) as e_sc, \
             tc.tile_pool(name="e_ps", bufs=2, space="PSUM") as e_ps:
            for c in range(C):
                wt = e_w.tile([P, len(VCH), H], BF16, tag="wt")
                nc.sync.dma_start(
                    out=wt[:, :7, :],
                    in_=embW[c, :896, :].rearrange("(k p) h -> p k h", p=P))
                nc.sync.dma_start(out=wt[:104, 7, :], in_=embW[c, 896:, :])
                for bt in range(BS // P):
                    bsl = slice(bt * P, (bt + 1) * P)
                    xt = e_x.tile([P, len(VCH), P], BF16, tag="xt")
                    nc.sync.dma_start(
                        out=xt[:, :7, :],
                        in_=xTin[c, :896, bsl].rearrange("(k p) b -> p k b", p=P))
                    nc.sync.dma_start(out=xt[:104, 7, :], in_=xTin[c, 896:, bsl])
                    ut = e_sc.tile([P, 1], F32, tag="ut")
                    wt15 = e_sc.tile([P, 1], F32, tag="wt15")
                    nc.sync.dma_start(out=ut[:], in_=uemb[c, bt, :, :])
                    nc.sync.dma_start(out=wt15[:], in_=w15emb[c, bt, :, :])
                    eps = e_ps.tile([P, H], F32, tag="eps")
                    for k, (v0, vn) in enumerate(VCH):
                        nc.tensor.matmul(eps[:], lhsT=xt[:vn, k, :], rhs=wt[:vn, k, :],
                                         start=(k == 0), stop=(k == len(VCH) - 1))
                    x0 = e_x.tile([P, H], F32, tag="x0")
                    # x0 = e*u + 15*(1-u)
                    nc.vector.tensor_scalar(out=x0[:], in0=eps[:], scalar1=ut[:],
                                            scalar2=wt15[:], op0=ALU.mult, op1=ALU.add)
                    nc.sync.dma_start(out=x_c[bsl, c, :], in_=x0[:])
                    if DEBUG:
                        nc.sync.dma_start(
                            out=dbg.rearrange("d (n c) h -> d n c h", c=C)[0, bsl, c, :],
                            in_=x0[:])

        # ---------------- transformer layers ----------------
        TBT = 2           # token tiles per block
        TB = TBT * P      # 256 tokens per block
        NB = T // TB      # 32 blocks
        for l in range(L):
            with tc.tile_pool(name="wpool", bufs=1) as wp, \
                 tc.tile_pool(name="xp", bufs=3) as xp, \
                 tc.tile_pool(name="hp", bufs=3) as hp, \
                 tc.tile_pool(name="qkp", bufs=3) as qkp, \
                 tc.tile_pool(name="ap", bufs=3) as ap_, \
                 tc.tile_pool(name="sp", bufs=6) as sp_, \
                 tc.tile_pool(name="gp", bufs=3) as gp, \
                 tc.tile_pool(name="mp", bufs=3) as mp, \
                 tc.tile_pool(name="ps_big", bufs=2, space="PSUM") as ps_big, \
                 tc.tile_pool(name="ps_med", bufs=2, space="PSUM") as ps_med, \
                 tc.tile_pool(name="ps_sm", bufs=4, space="PSUM") as ps_sm:
                ps_o = ps_sm
                wq_s = wp.tile([P, HC, H], BF16)
                wk_s = wp.tile([P, HC, H], BF16)
                wv_s = wp.tile([P, HC, H], BF16)
                wo_s = wp.tile([P, HC, H], BF16)
                w1_s = wp.tile([P, HC, FF], BF16)
                w2_s = wp.tile([P, FC, H], BF16)
                for wt_, src in ((wq_s, wq), (wk_s, wk), (wv_s, wv), (wo_s, wo),
                                 (w1_s, w1), (w2_s, w2)):
                    nc.sync.dma_start(
                        out=wt_[:],
                        in_=src[l].rearrange("(k p) n -> p k n", p=P))

                for blk in range(NB):
                    t0 = blk * TBT  # first token-tile index
                    tok0 = blk * TB
                    xs = xp.tile([P, TBT, H], F32, tag="xs")
                    nc.sync.dma_start(
                        out=xs[:],
                        in_=xbuf[tok0:tok0 + TB, :].rearrange("(t p) h -> p t h", p=P))

                    # LN1 -> h (bf16) ; hT via PE transpose (bf16)
                    h = hp.tile([P, TBT, H], BF16, tag="h")
                    hT = hp.tile([P, HC, TB], BF16, tag="hT")
                    for i in range(TBT):
                        _layernorm(nc, sp_, xs[:, i, :], h[:, i, :], eps_t)
                    for i in range(TBT):
                        for kc in range(HC):
                            tp = ps_sm.tile([P, P], BF16, tag="sm")
                            nc.tensor.transpose(tp[:], in_=h[:, i, kc * P:(kc + 1) * P],
                                                identity=ident[:])
                            nc.vector.tensor_copy(out=hT[:, kc, i * P:(i + 1) * P], in_=tp[:])

                    # qT, kT feature-major [P(hout), HC, TB]
                    qT = qkp.tile([P, HC, TB], BF16, tag="qT")
                    kT = qkp.tile([P, HC, TB], BF16, tag="kT")
                    for dst, wmat in ((qT, wq_s), (kT, wk_s)):
                        for ho in range(HC):
                            pq = ps_med.tile([P, TB], F32, tag="med")
                            for ki in range(HC):
                                nc.tensor.matmul(pq[:], lhsT=wmat[:, ki, ho * P:(ho + 1) * P],
                                                 rhs=hT[:, ki, :],
                                                 start=(ki == 0), stop=(ki == HC - 1))
                            nc.vector.tensor_copy(out=dst[:, ho, :], in_=pq[:])

                    # v token-major [P(tok), TBT, H]
                    v_s = ap_.tile([P, TBT, H], BF16, tag="v_s")
                    for i in range(TBT):
                        pv = ps_big.tile([P, H], F32, tag="big")
                        for ki in range(HC):
                            nc.tensor.matmul(pv[:], lhsT=hT[:, ki, i * P:(i + 1) * P],
                                             rhs=wv_s[:, ki, :],
                                             start=(ki == 0), stop=(ki == HC - 1))
                        nc.vector.tensor_copy(out=v_s[:, i, :], in_=pv[:])

                    # attention per tile
                    for i in range(TBT):
                        vm = mp.tile([P, P], BF16, tag="vm")
                        am = mp.tile([P, P], BF16, tag="am")
                        nc.sync.dma_start(out=vm[:], in_=vmask[t0 + i, :, :])
                        nc.sync.dma_start(out=am[:], in_=amask[t0 + i, :, :])
                        oT = ap_.tile([P, HC, P], BF16, tag="oT")  # feature-major o
                        for hc2 in range(HC):    # head pair -> one psum tile
                            po = ps_o.tile([P, P], F32, tag="sm")
                            for par in range(2):
                                hh = hc2 * 2 + par
                                ho, po_off = hh // 2, (hh % 2) * 64
                                rr = slice((hh % 2) * 64, (hh % 2) * 64 + 64)
                                # rows of qT chunk for this head
                                qrow = slice((hh * DK) % P, (hh * DK) % P + DK)
                                qch = (hh * DK) // P
                                psc = ps_sm.tile([P, P], F32, tag="sm")
                                nc.tensor.matmul(psc[:], lhsT=qT[qrow, qch, i * P:(i + 1) * P],
                                                 rhs=kT[qrow, qch, i * P:(i + 1) * P],
                                                 start=True, stop=True)
                                s2 = sp_.tile([P, P], F32, tag="s2")
                                nc.vector.tensor_tensor(out=s2[:], in0=psc[:], in1=vm[:],
                                                        op=ALU.mult)
                                nc.vector.tensor_tensor(out=s2[:], in0=s2[:], in1=am[:],
                                                        op=ALU.add)
                                mx = sp_.tile([P, 1], F32, tag="mx")
                                nc.vector.tensor_reduce(out=mx[:], in_=s2[:], axis=AX.X,
                                                        op=ALU.max)
                                nmx = sp_.tile([P, 1], F32, tag="nmx")
                                nc.vector.tensor_scalar(out=nmx[:], in0=mx[:], scalar1=-1.0,
                                                        scalar2=None, op0=ALU.mult)
                                et = sp_.tile([P, P], F32, tag="et")
                                sm = sp_.tile([P, 1], F32, tag="sm")
                                nc.scalar.activation(out=et[:], in_=s2[:], func=AF.Exp,
                                                     bias=nmx[:], scale=1.0,
                                                     accum_out=sm[:])
                                rs = sp_.tile([P, 1], F32, tag="rs")
                                nc.vector.reciprocal(out=rs[:], in_=sm[:])
                                en = sp_.tile([P, P], BF16, tag="en")
                                nc.vector.tensor_scalar(out=en[:], in0=et[:], scalar1=rs[:],
                                                        scalar2=None, op0=ALU.mult)
                                pet = ps_sm.tile([P, P], BF16, tag="sm")
                                nc.tensor.transpose(pet[:], in_=en[:], identity=ident[:])
                                ets = sp_.tile([P, P], BF16, tag="ets")
                                nc.vector.tensor_copy(out=ets[:], in_=pet[:])
                                nc.tensor.matmul(po[po_off:po_off + 64, :],
                                                 lhsT=v_s[:, i, hh * DK:(hh + 1) * DK],
                                                 rhs=ets[:], start=True, stop=True)
                            nc.vector.tensor_copy(out=oT[:, hc2, :], in_=po[:])

                        # Wo + residual -> xs2 (f32)
                        pwo = ps_big.tile([P, H], F32, tag="big")
                        for kc in range(HC):
                            nc.tensor.matmul(pwo[:], lhsT=oT[:, kc, :], rhs=wo_s[:, kc, :],
                                             start=(kc == 0), stop=(kc == HC - 1))
                        nc.vector.tensor_tensor(out=xs[:, i, :], in0=pwo[:], in1=xs[:, i, :],
                                                op=ALU.add)

                    # LN2 -> h2, h2T
                    h2 = hp.tile([P, TBT, H], BF16, tag="h2")
                    h2T = hp.tile([P, HC, TB], BF16, tag="h2T")
                    for i in range(TBT):
                        _layernorm(nc, sp_, xs[:, i, :], h2[:, i, :], eps_t)
                    for i in range(TBT):
                        for kc in range(HC):
                            tp = ps_sm.tile([P, P], BF16, tag="sm")
                            nc.tensor.transpose(tp[:], in_=h2[:, i, kc * P:(kc + 1) * P],
                                                identity=ident[:])
                            nc.vector.tensor_copy(out=h2T[:, kc, i * P:(i + 1) * P], in_=tp[:])

                    # W1 (feature-major out) + GELU -> gT bf16 [P, FC, TB]
                    gT = gp.tile([P, FC, TB], BF16, tag="gT")
                    for fo in range(FC):
                        pg = ps_med.tile([P, TB], F32, tag="med")
                        for ki in range(HC):
                            nc.tensor.matmul(pg[:], lhsT=w1_s[:, ki, fo * P:(fo + 1) * P],
                                             rhs=h2T[:, ki, :],
                                             start=(ki == 0), stop=(ki == HC - 1))
                        nc.scalar.activation(out=gT[:, fo, :], in_=pg[:],
                                             func=AF.Gelu_apprx_tanh)

                    # W2 (token-major out) + residual -> write x
                    for i in range(TBT):
                        pw2 = ps_big.tile([P, H], F32, tag="big")
                        for kf in range(FC):
                            nc.tensor.matmul(pw2[:], lhsT=gT[:, kf, i * P:(i + 1) * P],
                                             rhs=w2_s[:, kf, :],
                                             start=(kf == 0), stop=(kf == FC - 1))
                        xo = xp.tile([P, H], F32, tag="xo")
                        nc.vector.tensor_tensor(out=xo[:], in0=pw2[:], in1=xs[:, i, :],
                                                op=ALU.add)
                        nc.sync.dma_start(
                            out=xbuf[tok0 + i * P:tok0 + (i + 1) * P, :], in_=xo[:])
                        if DEBUG:
                            nc.sync.dma_start(
                                out=dbg[1 + l, tok0 + i * P:tok0 + (i + 1) * P, :],
                                in_=xo[:])

        # ---------------- head phase ----------------
        with tc.tile_pool(name="h_w", bufs=2) as h_w, \
             tc.tile_pool(name="h_x", bufs=3) as h_x, \
             tc.tile_pool(name="h_s", bufs=3) as h_s, \
             tc.tile_pool(name="h_ps", bufs=2, space="PSUM") as h_ps, \
             tc.tile_pool(name="h_pt", bufs=2, space="PSUM") as h_pt:
            for c in range(C):
                hw = h_w.tile([P, HC, V], BF16, tag="hw")
                nc.sync.dma_start(out=hw[:],
                                  in_=headW[c].rearrange("(k p) v -> p k v", p=P))
                for bt in range(BS // P):
                    bsl = slice(bt * P, (bt + 1) * P)
                    xc = h_x.tile([P, H], F32, tag="xc")
                    nc.sync.dma_start(out=xc[:], in_=x_c[bsl, c, :])
                    xcb = h_x.tile([P, H], BF16, tag="xcb")
                    nc.vector.tensor_copy(out=xcb[:], in_=xc[:])
                    xcT = h_x.tile([P, HC, P], BF16, tag="xcT")
                    for kc in range(HC):
                        tp = h_pt.tile([P, P], BF16, tag="tp2")
                        nc.tensor.transpose(tp[:], in_=xcb[:, kc * P:(kc + 1) * P],
                                            identity=ident[:])
                        nc.vector.tensor_copy(out=xcT[:, kc, :], in_=tp[:])
                    lg = h_s.tile([P, V], F32, tag="hlg")
                    for ng in range(2):
                        nsl = slice(ng * 500, (ng + 1) * 500)
                        pl = h_ps.tile([P, 512], F32, tag="pl")
                        for ki in range(HC):
                            nc.tensor.matmul(pl[:, :500], lhsT=xcT[:, ki, :],
                                             rhs=hw[:, ki, nsl],
                                             start=(ki == 0), stop=(ki == HC - 1))
                        nc.vector.tensor_copy(out=lg[:, nsl], in_=pl[:, :500])
                    # log_softmax over V
                    mx = h_s.tile([P, 1], F32, tag="hmx")
                    nc.vector.tensor_reduce(out=mx[:], in_=lg[:], axis=AX.X, op=ALU.max)
                    nmx = h_s.tile([P, 1], F32, tag="hnmx")
                    nc.vector.tensor_scalar(out=nmx[:], in0=mx[:], scalar1=-1.0,
                                            scalar2=None, op0=ALU.mult)
                    ex = h_s.tile([P, V], F32, tag="hex")
                    nc.scalar.activation(out=ex[:], in_=lg[:], func=AF.Exp,
                                         bias=nmx[:], scale=1.0)
                    sm = h_s.tile([P, 1], F32, tag="hsm")
                    nc.vector.tensor_reduce(out=sm[:], in_=ex[:], axis=AX.X, op=ALU.add)
                    lnz = h_s.tile([P, 1], F32, tag="hlnz")
                    nc.scalar.activation(out=lnz[:], in_=sm[:], func=AF.Ln)
                    off = h_s.tile([P, 1], F32, tag="hoff")
                    nc.vector.tensor_tensor(out=off[:], in0=nmx[:], in1=lnz[:],
                                            op=ALU.subtract)
                    lo = h_s.tile([P, V], F32, tag="hlo")
                    nc.scalar.activation(out=lo[:], in_=lg[:], func=AF.Identity,
                                         bias=off[:], scale=1.0)
                    nc.sync.dma_start(out=out[bsl, c, :], in_=lo[:])

        const_cm.__exit__(None, None, None)

    nc.finalize()
    return nc


def _layernorm(nc, pool, x_ap, h_ap, eps_t):
    """h = (x - mean) / sqrt(var + eps); gains/betas are identity."""
    stats = pool.tile([P, 6], F32, tag="ln_st")
    nc.vector.bn_stats(out=stats[:], in_=x_ap)
    mv = pool.tile([P, 2], F32, tag="ln_mv")
    nc.vector.bn_aggr(out=mv[:], in_=stats[:])
    rstd = pool.tile([P, 1], F32, tag="ln_rs")
    nc.scalar.activation(out=rstd[:], in_=mv[:, 1:2], func=AF.Sqrt,
                         bias=eps_t[:], scale=1.0)
    nc.vector.reciprocal(out=rstd[:], in_=rstd[:])
    nmb = pool.tile([P, 1], F32, tag="ln_nm")
    nc.vector.tensor_tensor(out=nmb[:], in0=mv[:, 0:1], in1=rstd[:], op=ALU.mult)
    nc.vector.tensor_scalar(out=nmb[:], in0=nmb[:], scalar1=-1.0, scalar2=None,
                            op0=ALU.mult)
    nc.scalar.activation(out=h_ap, in_=x_ap, func=AF.Identity, bias=nmb[:], scale=rstd[:])


def kernel(**inputs):
    inp = inputs
    # identity-params fast path: all biases zero, LN gains 1 / betas 0
    for name in ("embed_b", "bq", "bk", "bv", "bo", "b1", "b2", "head_b",
                 "ln1_b", "ln2_b"):
        assert not np.any(inp[name]), f"nonzero {name} unsupported"
    assert np.all(inp["ln1_g"] == 1.0) and np.all(inp["ln2_g"] == 1.0)

    if "nc" not in _CACHED:
        _CACHED["nc"] = build_kernel()
    nc = _CACHED["nc"]

    bf = lambda a: np.ascontiguousarray(a).astype(BF16NP)
    u_full = (inp["masked_position"] == 0).astype(np.float32)        # [B, C]

    # per-tile attention masks
    D = np.kron(np.eye(P // C, dtype=np.float32), np.ones((C, C), np.float32))
    shared = {
        "embW": bf(inp["embed_W"]),
        "wq": bf(inp["Wq"]), "wk": bf(inp["Wk"]),
        "wv": bf(inp["Wv"]), "wo": bf(inp["Wo"]),
        "w1": bf(inp["W1"]), "w2": bf(inp["W2"]),
        "headW": bf(inp["head_W"]),
    }

    in_maps = []
    for r in range(NCORES):
        bsl = slice(r * BS, (r + 1) * BS)
        u = u_full[bsl]                                   # [BS, C]
        uf = u.reshape(-1)                                # [T]
        ut = uf.reshape(NT, P)
        outer = ut[:, :, None] * ut[:, None, :]           # [NT, P, P]
        vm = (D[None] * outer * SCALE).astype(BF16NP)
        am = (NEG_BIG * (D[None] - D[None] * outer) +
              NEG_HUGE * (1.0 - D[None])).astype(BF16NP)
        # u / 15*(1-u) indexed [c, bt, bl] with token rows (bt*128+bl)*16+c
        u_cb = u.reshape(BS // P, P, C).transpose(2, 0, 1)[..., None]
        m = dict(shared)
        m["xTin"] = bf(inp["inputs"][bsl].transpose(1, 2, 0))
        m["uemb"] = np.ascontiguousarray(u_cb.astype(np.float32))
        m["w15emb"] = np.ascontiguousarray((15.0 * (1.0 - u_cb)).astype(np.float32))
        m["vmask"] = np.ascontiguousarray(vm)
        m["amask"] = np.ascontiguousarray(am)
        in_maps.append(m)

    res = run_bass_kernel_spmd(nc, in_maps, core_ids=list(range(NCORES)))
    return np.concatenate([r["out"] for r in res.results], axis=0)

